# revision 11
# baseline (speedup 1.0000x reference)
"""Trainium2 Bass kernel for nn_CMTLiDAR (topk_masking).

Strategy (hardcoded for L=6, B=2, N=4096, C=512, NC=3, TOPK=120, 8 cores):

Only decoder layer -1 contributes to the output, so the 6-layer einsum
collapses to one layer.  Further, the center/height heads are only needed
for the 120 top-k rows per batch, so only the cls head runs on all rows.

Per-core roles: core c handles batch b = c//4, row chunk j = c%4
(rows [j*1024,(j+1)*1024) of that batch).

Pipeline per core:
  1. hidden2^T = relu(Wh2^T @ x^T + bh2)  (fp32 GEMM, rows sharded 4-way)
  2. cls logits -> softmax -> class-0 score for its 1024 rows
  3. AllGather scores within the 4-core batch group -> full 4096 scores
  4. kth_largest (gpsimd) -> exact 120th-largest threshold T
  5. mask + sparse_gather -> compacted candidate (idx, score) pairs
  6. pairwise rank (PE broadcast + DVE compares) with jax tie semantics
  7. indirect-DMA scatter by rank -> sorted top-120 (score, idx) in DRAM
  8. each core gathers its 30 output rows: x rows, ref points; computes
     center/height heads for those rows only; writes 30-row output slices
Host: shards/transposes inputs, concatenates the 8 core outputs.
"""

import numpy as np

import concourse.bass as bass
import concourse.bacc as bacc
import concourse.tile as tile
import concourse.mybir as mybir
from concourse.bass import ts, ds
from concourse.bass_utils import run_bass_kernel_spmd

F32 = mybir.dt.float32
I32 = mybir.dt.int32
U32 = mybir.dt.uint32
AF = mybir.ActivationFunctionType
OP = mybir.AluOpType

P = 128
C = 512
KT = C // P          # 4 k-tiles
B = 2
N = 4096
NC_CLS = 3
TOPK = 120
NCORES = 8
GROUP = 4            # cores per batch
RC = N // GROUP      # rows per core = 1024
OUTR = TOPK // GROUP  # output rows per core = 30

EPS = 1e-5
ONE_M_EPS = float(np.float32(1.0) - np.float32(1e-5))
NEG_BIG = -1.0e30


def emit(nc):
    """Emit the SPMD kernel body. Declares all DRAM I/O on nc."""
    # ---------------- inputs ----------------
    xT = nc.dram_tensor("xT", [C, RC], F32, kind="ExternalInput")
    xfull = nc.dram_tensor("xfull", [N, C], F32, kind="ExternalInput")
    refb = nc.dram_tensor("refb", [N, 3], F32, kind="ExternalInput")
    wh0 = nc.dram_tensor("wh0", [C, C], F32, kind="ExternalInput")
    wh1 = nc.dram_tensor("wh1", [C, C], F32, kind="ExternalInput")
    wh2 = nc.dram_tensor("wh2", [C, C], F32, kind="ExternalInput")
    bh0 = nc.dram_tensor("bh0", [C], F32, kind="ExternalInput")
    bh1 = nc.dram_tensor("bh1", [C], F32, kind="ExternalInput")
    bh2 = nc.dram_tensor("bh2", [C], F32, kind="ExternalInput")
    wcen = nc.dram_tensor("wcen", [C, 2], F32, kind="ExternalInput")
    bcen = nc.dram_tensor("bcen", [2], F32, kind="ExternalInput")
    whei = nc.dram_tensor("whei", [C, 1], F32, kind="ExternalInput")
    bhei = nc.dram_tensor("bhei", [1], F32, kind="ExternalInput")
    wcls = nc.dram_tensor("wcls", [C, NC_CLS], F32, kind="ExternalInput")
    bcls = nc.dram_tensor("bcls", [NC_CLS], F32, kind="ExternalInput")
    rowsel = nc.dram_tensor("rowsel", [OUTR, 1], I32, kind="ExternalInput")
    ident = nc.dram_tensor("ident", [P, P], F32, kind="ExternalInput")
    iota16 = nc.dram_tensor("iota16", [16, 256], F32, kind="ExternalInput")
    iotac0 = nc.dram_tensor("iotac0", [P, 1], F32, kind="ExternalInput")
    iotac1 = nc.dram_tensor("iotac1", [P, 1], F32, kind="ExternalInput")
    iotam = nc.dram_tensor("iotam", [P, 2 * P], F32, kind="ExternalInput")
    p16sel = nc.dram_tensor("p16sel", [16, P], F32, kind="ExternalInput")
    colsel = nc.dram_tensor("colsel", [P, 32], F32, kind="ExternalInput")
    iotar2 = nc.dram_tensor("iotar2", [1, 2 * P], F32, kind="ExternalInput")
    bselc = nc.dram_tensor("bselc", [P, 1], F32, kind="ExternalInput")
    bsel16 = nc.dram_tensor("bsel16", [16, 1], F32, kind="ExternalInput")
    # ---------------- outputs ----------------
    qry = nc.dram_tensor("qry", [OUTR, C], F32, kind="ExternalOutput")
    refq = nc.dram_tensor("refq", [OUTR, 3], F32, kind="ExternalOutput")
    cen = nc.dram_tensor("cen", [OUTR, 2], F32, kind="ExternalOutput")
    hei = nc.dram_tensor("hei", [OUTR, 1], F32, kind="ExternalOutput")
    sco = nc.dram_tensor("sco", [OUTR, 1], F32, kind="ExternalOutput")

    with tile.TileContext(nc) as tc:
        with tc.tile_pool(name="const", bufs=1) as cst, \
             tc.tile_pool(name="work", bufs=1) as wk, \
             tc.tile_pool(name="psum_mm", bufs=4, space="PSUM") as pmm, \
             tc.tile_pool(name="psum_sm", bufs=2, space="PSUM") as psm, \
             tc.tile_pool(name="dram", bufs=1, space="DRAM") as dr:

            # ======== load constants / weights ========
            ones1 = cst.tile([1, P], F32)
            nc.vector.memset(ones1[:], 1.0)
            ones_c = cst.tile([P, 1], F32)
            nc.vector.memset(ones_c[:], 1.0)

            # PE warm-up: small matmuls with no data deps to ramp the HAM
            # clock while the input DMAs stream in.
            warm_ps = psm.tile([P, 256], F32, tag="sm")
            for _ in range(8):
                nc.tensor.matmul(warm_ps[:, :P], lhsT=ones1[:], rhs=ones1[:],
                                 start=True, stop=True)
            warm_sb = wk.tile([1, 1], F32)
            nc.vector.tensor_copy(warm_sb[:], warm_ps[0:1, 0:1])
            warmd = dr.tile([1, 1], F32)
            nc.sync.dma_start(warmd[:], warm_sb[:])

            wh2_sb = cst.tile([P, KT, C], F32)
            xT_sb = wk.tile([P, KT, RC], F32)
            for ko in range(KT):
                for mo in range(KT):
                    nc.sync.dma_start(wh2_sb[:, ko, ts(mo, P)],
                                      wh2[ts(ko, P), ts(mo, P)])
                for no in range(RC // 512):
                    nc.scalar.dma_start(xT_sb[:, ko, ts(no, 512)],
                                        xT[ts(ko, P), ts(no, 512)])
            bh2_sb = cst.tile([P, KT], F32)
            nc.sync.dma_start(bh2_sb[:], bh2[:].rearrange("(mo p) -> p mo", p=P))
            wcls_sb = cst.tile([P, KT, NC_CLS], F32)
            nc.sync.dma_start(wcls_sb[:], wcls[:].rearrange("(ko p) n -> p ko n", p=P))
            bcls_sb = cst.tile([1, NC_CLS], F32)
            nc.sync.dma_start(bcls_sb[:], bcls[:].rearrange("(o n) -> o n", o=1))
            ident_sb = cst.tile([P, P], F32)
            nc.sync.dma_start(ident_sb[:], ident[:])

            # ======== stage 1: hidden2T = relu(Wh2^T x^T + bh2) ========
            h2T = wk.tile([P, KT, RC], F32)   # [d-part, d-tile, row]
            for mo in range(KT):
                pss = [pmm.tile([P, 512], F32, tag="mm", name=f"ps{mo}_{i}") for i in range(2)]
                for ko in range(KT):
                    for no in range(RC // 512):
                        nc.tensor.matmul(
                            pss[no][:],
                            lhsT=wh2_sb[:, ko, ts(mo, P)],
                            rhs=xT_sb[:, ko, ts(no, 512)],
                            start=(ko == 0), stop=(ko == KT - 1),
                        )
                for no in range(RC // 512):
                    dst = h2T[:, mo, ts(no, 512)]
                    if no % 2 == 0:
                        nc.scalar.activation(dst, pss[no][:], AF.Relu,
                                             bias=bh2_sb[:, mo:mo + 1])
                    else:
                        nc.vector.tensor_scalar(
                            out=dst, in0=pss[no][:],
                            scalar1=bh2_sb[:, mo:mo + 1], scalar2=0.0,
                            op0=OP.add, op1=OP.max)

            # ======== stage 1b: cls logits + softmax -> scores ========
            l_sb = wk.tile([P, 8, NC_CLS], F32)
            for ro in range(8):
                pl = psm.tile([P, 256], F32, tag="sm")
                for ko2 in range(KT):
                    nc.tensor.matmul(
                        pl[:, :NC_CLS],
                        lhsT=h2T[:, ko2, ts(ro, P)],
                        rhs=wcls_sb[:, ko2],
                        start=(ko2 == 0), stop=(ko2 == KT - 1))
                nc.vector.tensor_copy(l_sb[:, ro], pl[:, :NC_CLS])

            mx = wk.tile([P, 8], F32)
            nc.vector.tensor_reduce(mx[:], l_sb[:], axis=mybir.AxisListType.X,
                                    op=OP.max)
            lc = wk.tile([P, 8, NC_CLS], F32)
            nc.vector.tensor_tensor(
                out=lc[:], in0=l_sb[:],
                in1=mx[:, :, None].to_broadcast([P, 8, NC_CLS]),
                op=OP.subtract)
            bcb_ps = psm.tile([P, 256], F32, tag="sm")
            nc.tensor.matmul(bcb_ps[:, :NC_CLS], lhsT=ones1[:], rhs=bcls_sb[:],
                             start=True, stop=True)
            bcls_bc = wk.tile([P, NC_CLS], F32)
            nc.vector.tensor_copy(bcls_bc[:], bcb_ps[:, :NC_CLS])
            nc.vector.tensor_tensor(
                out=lc[:], in0=lc[:],
                in1=bcls_bc[:, None, :].to_broadcast([P, 8, NC_CLS]),
                op=OP.add)
            e_sb = wk.tile([P, 8, NC_CLS], F32)
            nc.scalar.activation(e_sb[:], lc[:], AF.Exp)
            ssum = wk.tile([P, 8], F32)
            nc.vector.tensor_reduce(ssum[:], e_sb[:], axis=mybir.AxisListType.X,
                                    op=OP.add)
            rs = wk.tile([P, 8], F32)
            nc.vector.reciprocal(rs[:], ssum[:])
            score_sb = wk.tile([P, 8], F32)
            nc.vector.tensor_tensor(out=score_sb[:], in0=e_sb[:, :, 0],
                                    in1=rs[:], op=OP.mult)

            # ======== stage 1c: allgather scores over batch group ========
            scl = dr.tile([RC], F32)
            nc.sync.dma_start(scl[:].rearrange("(ro p) -> p ro", p=P), score_sb[:])
            scg_t = nc.dram_tensor("scg_shared", [B * N], F32, kind="Internal",
                                   addr_space="Shared")
            scg = scg_t.ap()
            nc.gpsimd.collective_compute(
                "AllGather", OP.bypass,
                replica_groups=[list(range(NCORES))],
                ins=[scl.opt()], outs=[scg])

            # ======== stage 2: threshold = 120th largest of the union of
            # per-partition top-2 (a lower bound on the exact 120th-largest
            # score; the final ranker below restores exactness) ========
            s128b = wk.tile([P, B, N // P], F32)
            nc.sync.dma_start(s128b[:],
                              scg[:].rearrange("(b p f) -> p b f", b=B, p=P))
            s16b = wk.tile([16, B, N // 16], F32)
            nc.scalar.dma_start(s16b[:],
                                scg[:].rearrange("(b f p) -> p b f", b=B, p=16))
            bselc_sb = cst.tile([P, 1], F32)
            nc.sync.dma_start(bselc_sb[:], bselc[:])
            bsel16_sb = cst.tile([16, 1], F32)
            nc.sync.dma_start(bsel16_sb[:], bsel16[:])
            # blend the two batches: s = s0*(1-b) + s1*b = s0 + (s1-s0)*b
            s128 = wk.tile([P, N // P], F32)
            nc.vector.tensor_tensor(out=s128[:], in0=s128b[:, 1],
                                    in1=s128b[:, 0], op=OP.subtract)
            nc.vector.tensor_scalar(out=s128[:], in0=s128[:],
                                    scalar1=bselc_sb[:, 0:1], scalar2=None,
                                    op0=OP.mult)
            nc.vector.tensor_tensor(out=s128[:], in0=s128[:],
                                    in1=s128b[:, 0], op=OP.add)
            s16 = wk.tile([16, N // 16], F32)
            nc.vector.tensor_tensor(out=s16[:], in0=s16b[:, 1],
                                    in1=s16b[:, 0], op=OP.subtract)
            nc.vector.tensor_scalar(out=s16[:], in0=s16[:],
                                    scalar1=bsel16_sb[:, 0:1], scalar2=None,
                                    op0=OP.mult)
            nc.vector.tensor_tensor(out=s16[:], in0=s16[:],
                                    in1=s16b[:, 0], op=OP.add)
            iotam_sb = cst.tile([P, 2 * P], F32)
            nc.sync.dma_start(iotam_sb[:], iotam[:])
            iotac_sb = [cst.tile([P, 1], F32, tag=f"iotac{c}", name=f"iotac{c}")
                        for c in range(2)]
            nc.sync.dma_start(iotac_sb[0][:], iotac0[:])
            nc.sync.dma_start(iotac_sb[1][:], iotac1[:])
            # [m < j] tie-break masks, shared by both rankers
            mlt = []
            for c in range(2):
                ml = wk.tile([P, 2 * P], F32, tag=f"mlt{c}", name=f"mlt{c}")
                nc.vector.tensor_scalar(out=ml[:], in0=iotam_sb[:],
                                        scalar1=iotac_sb[c][:, 0:1],
                                        scalar2=None, op0=OP.is_lt)
                mlt.append(ml)

            m8 = wk.tile([P, 8], F32)
            nc.vector.max(m8[:], s128[:])
            # union candidate m = 128*f + p holds m8[p, f] (f in {0,1})
            u_row = wk.tile([1, 2 * P], F32)
            nc.sync.dma_start(u_row[:, 0:P], m8[:, 0:1])
            nc.scalar.dma_start(u_row[:, P:2 * P], m8[:, 1:2])
            bc2 = psm.tile([P, 512], F32, tag="sm")
            nc.tensor.matmul(bc2[:, :2 * P], lhsT=ones1[:], rhs=u_row[:],
                             start=True, stop=True)
            tv = []
            for c in range(2):
                gtc = wk.tile([P, 2 * P], F32, tag=f"gtc{c}", name=f"gtc{c}")
                nc.vector.tensor_scalar(out=gtc[:], in0=bc2[:, :2 * P],
                                        scalar1=m8[:, c:c + 1], scalar2=None,
                                        op0=OP.is_gt)
                eqc = wk.tile([P, 2 * P], F32, tag=f"eqc{c}", name=f"eqc{c}")
                nc.vector.tensor_scalar(out=eqc[:], in0=bc2[:, :2 * P],
                                        scalar1=m8[:, c:c + 1], scalar2=None,
                                        op0=OP.is_equal)
                nc.vector.tensor_tensor(out=eqc[:], in0=eqc[:], in1=mlt[c][:],
                                        op=OP.mult)
                nc.vector.tensor_tensor(out=gtc[:], in0=gtc[:], in1=eqc[:],
                                        op=OP.add)
                rkc = wk.tile([P, 1], F32, tag=f"rkc{c}", name=f"rkc{c}")
                nc.vector.tensor_reduce(rkc[:], gtc[:],
                                        axis=mybir.AxisListType.X, op=OP.add)
                selc = wk.tile([P, 1], F32, tag=f"selc{c}", name=f"selc{c}")
                nc.vector.tensor_scalar(out=selc[:], in0=rkc[:],
                                        scalar1=float(TOPK - 1), scalar2=None,
                                        op0=OP.is_equal)
                nc.vector.tensor_tensor(out=selc[:], in0=selc[:],
                                        in1=m8[:, c:c + 1], op=OP.mult)
                tv.append(selc)
            nc.vector.tensor_tensor(out=tv[0][:], in0=tv[0][:], in1=tv[1][:],
                                    op=OP.add)
            t_ps = psm.tile([P, 256], F32, tag="sm")
            nc.tensor.matmul(t_ps[:1, :1], lhsT=tv[0][:], rhs=ones_c[:],
                             start=True, stop=True)
            t11 = wk.tile([1, 1], F32)
            nc.vector.tensor_copy(t11[:], t_ps[0:1, 0:1])
            t16p = psm.tile([P, 256], F32, tag="sm")
            nc.tensor.matmul(t16p[:16, :1], lhsT=ones1[0:1, 0:16], rhs=t11[:],
                             start=True, stop=True)
            t16 = wk.tile([16, 1], F32)
            nc.vector.tensor_copy(t16[:], t16p[:16, :1])

            # ======== stage 2b: mask + compact candidates (>= T) ========
            iota16_sb = cst.tile([16, 256], F32)
            nc.sync.dma_start(iota16_sb[:], iota16[:])
            neg16 = cst.tile([16, 256], F32)
            nc.vector.memset(neg16[:], -1.0)
            mask16 = wk.tile([16, 256], U32)
            nc.vector.tensor_scalar(out=mask16[:], in0=s16[:],
                                    scalar1=t16[:, 0:1], scalar2=None,
                                    op0=OP.is_ge)
            ms = wk.tile([16, 256], F32)
            nc.vector.select(ms[:], mask16[:], s16[:], neg16[:])
            mi = wk.tile([16, 256], F32)
            nc.vector.select(mi[:], mask16[:], iota16_sb[:], neg16[:])

            NCAND = 2 * P
            csco16 = wk.tile([16, 16], F32)
            nf1 = wk.tile([1, 1], U32)
            nc.gpsimd.sparse_gather(csco16[:], ms[:], num_found=nf1[:])
            cidx16 = wk.tile([16, 16], F32)
            nf2 = wk.tile([1, 1], U32)
            nc.gpsimd.sparse_gather(cidx16[:], mi[:], num_found=nf2[:])

            # score row [1, 256] (candidate j = p + 16f) via PE transpose +
            # one sbuf-to-sbuf reshape DMA
            tpc = psm.tile([P, 256], F32, tag="sm")
            nc.tensor.transpose(tpc[:16, :16], csco16[:], ident_sb[:16, :16])
            cts = wk.tile([16, 16], F32)
            nc.vector.tensor_copy(cts[:], tpc[:16, :16])
            cs_row = wk.tile([1, NCAND], F32)
            nc.sync.dma_start(cs_row[:], cts[:])

            # per-partition columns via PE one-hot select + masked reduce
            p16sel_sb = cst.tile([16, P], F32)
            nc.sync.dma_start(p16sel_sb[:], p16sel[:])
            colsel_sb = cst.tile([P, 32], F32)
            nc.sync.dma_start(colsel_sb[:], colsel[:])
            cs_col, ci_col = [], []
            for arr, src16 in ((0, csco16), (1, cidx16)):
                xp = psm.tile([P, 256], F32, tag="sm", name=f"xp{arr}")
                nc.tensor.matmul(xp[:, :16], lhsT=p16sel_sb[:], rhs=src16[:],
                                 start=True, stop=True)
                for c in range(2):
                    tm = wk.tile([P, 16], F32, tag=f"tm{arr}{c}",
                                 name=f"tm{arr}{c}")
                    nc.vector.tensor_tensor(
                        out=tm[:], in0=xp[:, :16],
                        in1=colsel_sb[:, ts(c, 16)], op=OP.mult)
                    col = wk.tile([P, 1], F32, tag=f"col{arr}{c}",
                                  name=f"col{arr}{c}")
                    nc.vector.tensor_reduce(col[:], tm[:],
                                            axis=mybir.AxisListType.X,
                                            op=OP.add)
                    (cs_col if arr == 0 else ci_col).append(col)

            # validity masks (slots beyond num_found hold garbage)
            iotar_sb = cst.tile([1, NCAND], F32)
            nc.sync.dma_start(iotar_sb[:], iotar2[:])
            nf11f = wk.tile([1, 1], F32)
            nc.vector.tensor_copy(nf11f[:], nf1[:])
            nfp = psm.tile([P, 256], F32, tag="sm")
            nc.tensor.matmul(nfp[:, :1], lhsT=ones1[:], rhs=nf11f[:],
                             start=True, stop=True)
            nf128f = wk.tile([P, 1], F32)
            nc.vector.tensor_copy(nf128f[:], nfp[:, :1])
            validr = wk.tile([1, NCAND], U32)
            nc.vector.tensor_scalar(out=validr[:], in0=iotar_sb[:],
                                    scalar1=nf11f[:, 0:1], scalar2=None,
                                    op0=OP.is_lt)
            negr = cst.tile([1, NCAND], F32)
            nc.vector.memset(negr[:], NEG_BIG)
            csm_row = wk.tile([1, NCAND], F32)
            nc.vector.select(csm_row[:], validr[:], cs_row[:], negr[:])
            negc = cst.tile([P, 1], F32)
            nc.vector.memset(negc[:], NEG_BIG)
            csm_col = []
            for c in range(2):
                validc = wk.tile([P, 1], U32, tag=f"validc{c}",
                                 name=f"validc{c}")
                nc.vector.tensor_tensor(out=validc[:], in0=iotac_sb[c][:],
                                        in1=nf128f[:], op=OP.is_lt)
                cm = wk.tile([P, 1], F32, tag=f"csmcol{c}", name=f"csmcol{c}")
                nc.vector.select(cm[:], validc[:], cs_col[c][:], negc[:])
                csm_col.append(cm)

            # broadcast scores across partitions; rank with jax tie
            # semantics ([m < j] on equal scores); scatter by rank
            bc = psm.tile([P, 512], F32, tag="sm")
            nc.tensor.matmul(bc[:, :NCAND], lhsT=ones1[:], rhs=csm_row[:],
                             start=True, stop=True)
            sortd = dr.tile([TOPK, 2], F32)
            for c in range(2):
                gt = wk.tile([P, NCAND], F32, tag=f"gt{c}", name=f"gt{c}")
                nc.vector.tensor_scalar(out=gt[:], in0=bc[:, :NCAND],
                                        scalar1=csm_col[c][:, 0:1],
                                        scalar2=None, op0=OP.is_gt)
                eq = wk.tile([P, NCAND], F32, tag=f"eq{c}", name=f"eq{c}")
                nc.vector.tensor_scalar(out=eq[:], in0=bc[:, :NCAND],
                                        scalar1=csm_col[c][:, 0:1],
                                        scalar2=None, op0=OP.is_equal)
                nc.vector.tensor_tensor(out=eq[:], in0=eq[:], in1=mlt[c][:],
                                        op=OP.mult)
                nc.vector.tensor_tensor(out=gt[:], in0=gt[:], in1=eq[:],
                                        op=OP.add)
                rankf = wk.tile([P, 1], F32, tag=f"rankf{c}", name=f"rankf{c}")
                nc.vector.tensor_reduce(rankf[:], gt[:],
                                        axis=mybir.AxisListType.X, op=OP.add)
                ranki = wk.tile([P, 1], I32, tag=f"ranki{c}", name=f"ranki{c}")
                nc.vector.tensor_copy(ranki[:], rankf[:])
                pairs = wk.tile([P, 2], F32, tag=f"pairs{c}", name=f"pairs{c}")
                nc.vector.tensor_copy(pairs[:, 0:1], cs_col[c][:])
                nc.vector.tensor_copy(pairs[:, 1:2], ci_col[c][:])
                nc.gpsimd.indirect_dma_start(
                    out=sortd[:],
                    out_offset=bass.IndirectOffsetOnAxis(ap=ranki[:, 0:1],
                                                         axis=0),
                    in_=pairs[:], in_offset=None,
                    bounds_check=TOPK - 1, oob_is_err=False)

            # ======== stage 3: gather this core's 30 rows ========
            rsel = wk.tile([OUTR, 1], I32)
            nc.sync.dma_start(rsel[:], rowsel[:])
            pairs30 = wk.tile([OUTR, 2], F32)
            nc.gpsimd.indirect_dma_start(
                out=pairs30[:], out_offset=None,
                in_=sortd[:],
                in_offset=bass.IndirectOffsetOnAxis(ap=rsel[:, 0:1], axis=0))
            idx30 = wk.tile([OUTR, 1], I32)
            nc.vector.tensor_copy(idx30[:], pairs30[:, 1:2])
            xg = wk.tile([OUTR, C], F32)
            nc.gpsimd.indirect_dma_start(
                out=xg[:], out_offset=None,
                in_=xfull[:],
                in_offset=bass.IndirectOffsetOnAxis(ap=idx30[:, 0:1], axis=0))
            refg = wk.tile([OUTR, 3], F32)
            nc.gpsimd.indirect_dma_start(
                out=refg[:], out_offset=None,
                in_=refb[:],
                in_offset=bass.IndirectOffsetOnAxis(ap=idx30[:, 0:1], axis=0))

            # ======== stage 4: center/height heads on 30 rows ========
            xgT = wk.tile([P, KT, OUTR], F32)
            for kt in range(KT):
                tp = psm.tile([P, 256], F32, tag="sm")
                nc.tensor.transpose(tp[:, :OUTR], xg[:, ts(kt, P)],
                                    ident_sb[:OUTR, :OUTR])
                nc.vector.tensor_copy(xgT[:, kt], tp[:, :OUTR])

            hT = []
            for h, (whd, bhd) in enumerate([(wh0, bh0), (wh1, bh1)]):
                wh_sb = cst.tile([P, KT, C], F32, tag=f"wh{h}sb")
                nc.sync.dma_start(wh_sb[:],
                                  whd[:].rearrange("(ko p) d -> p ko d", p=P))
                bh_sb = cst.tile([P, KT], F32, tag=f"bh{h}sb")
                nc.sync.dma_start(bh_sb[:],
                                  bhd[:].rearrange("(mo p) -> p mo", p=P))
                hT_h = wk.tile([P, KT, OUTR], F32, tag=f"hT{h}")
                for mo in range(KT):
                    hp = psm.tile([P, 256], F32, tag="sm")
                    for ko in range(KT):
                        nc.tensor.matmul(
                            hp[:, :OUTR],
                            lhsT=wh_sb[:, ko, ts(mo, P)],
                            rhs=xgT[:, ko],
                            start=(ko == 0), stop=(ko == KT - 1))
                    nc.scalar.activation(hT_h[:, mo], hp[:, :OUTR], AF.Relu,
                                         bias=bh_sb[:, mo:mo + 1])
                hT.append(hT_h)

            wcen_sb = cst.tile([P, KT, 2], F32)
            nc.sync.dma_start(wcen_sb[:], wcen[:].rearrange("(ko p) n -> p ko n", p=P))
            bcen_sb = cst.tile([1, 2], F32)
            nc.sync.dma_start(bcen_sb[:], bcen[:].rearrange("(o n) -> o n", o=1))
            whei_sb = cst.tile([P, KT, 1], F32)
            nc.sync.dma_start(whei_sb[:], whei[:].rearrange("(ko p) n -> p ko n", p=P))
            bhei_sb = cst.tile([1, 1], F32)
            nc.sync.dma_start(bhei_sb[:], bhei[:].rearrange("(o n) -> o n", o=1))
            ones30 = cst.tile([1, OUTR], F32)
            nc.vector.memset(ones30[:], 1.0)

            cen_ps = psm.tile([P, 256], F32, tag="sm")
            for ko2 in range(KT):
                nc.tensor.matmul(cen_ps[:OUTR, :2], lhsT=hT[0][:, ko2],
                                 rhs=wcen_sb[:, ko2],
                                 start=(ko2 == 0), stop=False)
            nc.tensor.matmul(cen_ps[:OUTR, :2], lhsT=ones30[:], rhs=bcen_sb[:],
                             start=False, stop=True)
            hei_ps = psm.tile([P, 256], F32, tag="sm")
            for ko2 in range(KT):
                nc.tensor.matmul(hei_ps[:OUTR, :1], lhsT=hT[1][:, ko2],
                                 rhs=whei_sb[:, ko2],
                                 start=(ko2 == 0), stop=False)
            nc.tensor.matmul(hei_ps[:OUTR, :1], lhsT=ones30[:], rhs=bhei_sb[:],
                             start=False, stop=True)

            # ======== stage 5: epilogue ========
            rcl = wk.tile([OUTR, 3], F32)
            nc.vector.tensor_scalar(out=rcl[:], in0=refg[:],
                                    scalar1=EPS, scalar2=ONE_M_EPS,
                                    op0=OP.max, op1=OP.min)
            ln_r = wk.tile([OUTR, 3], F32)
            nc.scalar.activation(ln_r[:], rcl[:], AF.Ln)
            om = wk.tile([OUTR, 3], F32)
            nc.vector.tensor_scalar(out=om[:], in0=rcl[:],
                                    scalar1=-1.0, scalar2=1.0,
                                    op0=OP.mult, op1=OP.add)
            ln_om = wk.tile([OUTR, 3], F32)
            nc.scalar.activation(ln_om[:], om[:], AF.Ln)
            iv = wk.tile([OUTR, 3], F32)
            nc.vector.tensor_sub(iv[:], ln_r[:], ln_om[:])

            cr = wk.tile([OUTR, 2], F32)
            nc.vector.tensor_add(cr[:], cen_ps[:OUTR, :2], iv[:, 0:2])
            csg = wk.tile([OUTR, 2], F32)
            nc.scalar.activation(csg[:], cr[:], AF.Sigmoid)
            cxy = wk.tile([OUTR, 2], F32)
            nc.vector.tensor_scalar(out=cxy[:], in0=csg[:],
                                    scalar1=108.0, scalar2=-54.0,
                                    op0=OP.mult, op1=OP.add)
            hr = wk.tile([OUTR, 1], F32)
            nc.vector.tensor_add(hr[:], hei_ps[:OUTR, :1], iv[:, 2:3])
            hsg = wk.tile([OUTR, 1], F32)
            nc.scalar.activation(hsg[:], hr[:], AF.Sigmoid)
            hz = wk.tile([OUTR, 1], F32)
            nc.vector.tensor_scalar(out=hz[:], in0=hsg[:],
                                    scalar1=8.0, scalar2=-5.0,
                                    op0=OP.mult, op1=OP.add)

            # ======== outputs ========
            nc.sync.dma_start(qry[:], xg[:])
            nc.sync.dma_start(refq[:], refg[:])
            nc.sync.dma_start(cen[:], cxy[:])
            nc.sync.dma_start(hei[:], hz[:])
            nc.sync.dma_start(sco[:], pairs30[:, 0:1])


_CACHE = {}


def build():
    if "nc" not in _CACHE:
        nc = bacc.Bacc("TRN2", target_bir_lowering=False, debug=False,
                       num_devices=NCORES)
        emit(nc)
        nc.compile()
        _CACHE["nc"] = nc
    return _CACHE["nc"]


def make_in_maps(outs_dec, reference_points, Wh, bh, W_center, b_center,
                 W_height, b_height, W_cls, b_cls):
    x = np.ascontiguousarray(outs_dec[-1])          # [B, N, C]
    iota16 = (np.arange(256, dtype=np.float32)[None, :] * 16
              + np.arange(16, dtype=np.float32)[:, None])
    common = dict(
        wh0=np.ascontiguousarray(Wh[0]), wh1=np.ascontiguousarray(Wh[1]),
        wh2=np.ascontiguousarray(Wh[2]),
        bh0=np.ascontiguousarray(bh[0]), bh1=np.ascontiguousarray(bh[1]),
        bh2=np.ascontiguousarray(bh[2]),
        wcen=np.ascontiguousarray(W_center), bcen=np.ascontiguousarray(b_center),
        whei=np.ascontiguousarray(W_height), bhei=np.ascontiguousarray(b_height),
        wcls=np.ascontiguousarray(W_cls), bcls=np.ascontiguousarray(b_cls),
        ident=np.eye(P, dtype=np.float32),
        iota16=np.ascontiguousarray(iota16),
        iotac0=np.arange(P, dtype=np.float32)[:, None].copy(),
        iotac1=(P + np.arange(P, dtype=np.float32))[:, None].copy(),
        iotam=np.tile(np.arange(2 * P, dtype=np.float32), (P, 1)),
        p16sel=(np.arange(16)[:, None] == (np.arange(P)[None, :] % 16)
                ).astype(np.float32),
        colsel=np.concatenate(
            [(np.arange(16)[None, :] == (8 * c + np.arange(P)[:, None] // 16)
              ).astype(np.float32) for c in range(2)], axis=1),
        iotar2=np.arange(2 * P, dtype=np.float32)[None, :].copy(),
    )
    in_maps = []
    for c in range(NCORES):
        b, j = divmod(c, GROUP)
        xb = x[b]
        in_maps.append(dict(
            common,
            xT=np.ascontiguousarray(xb[j * RC:(j + 1) * RC].T),
            xfull=xb,
            refb=np.ascontiguousarray(reference_points[b]),
            rowsel=(j * OUTR + np.arange(OUTR, dtype=np.int32))[:, None].copy(),
            bselc=np.full((P, 1), float(b), np.float32),
            bsel16=np.full((16, 1), float(b), np.float32),
        ))
    return in_maps


def assemble(results):
    out = np.zeros((B, TOPK, C + 3 + 3 + 1), dtype=np.float32)
    for c in range(NCORES):
        b, j = divmod(c, GROUP)
        r = results[c]
        sl = slice(j * OUTR, (j + 1) * OUTR)
        out[b, sl, :C] = r["qry"]
        out[b, sl, C:C + 3] = r["refq"]
        out[b, sl, C + 3:C + 5] = r["cen"]
        out[b, sl, C + 5:C + 6] = r["hei"]
        out[b, sl, C + 6] = r["sco"][:, 0]
    return out


def kernel(**inputs):
    nc = build()
    in_maps = make_in_maps(**{k: np.asarray(v) for k, v in inputs.items()})
    res = run_bass_kernel_spmd(nc, in_maps, core_ids=list(range(NCORES)))
    return assemble(res.results)


if __name__ == "__main__":
    rng = np.random.default_rng(0)
    ins = dict(
        outs_dec=rng.standard_normal((6, B, N, C)).astype(np.float32),
        reference_points=rng.uniform(0.01, 0.99, (B, N, 3)).astype(np.float32),
        Wh=(rng.standard_normal((3, C, C)) * 0.02).astype(np.float32),
        bh=np.zeros((3, C), np.float32),
        W_center=(rng.standard_normal((C, 2)) * 0.02).astype(np.float32),
        b_center=np.zeros(2, np.float32),
        W_height=(rng.standard_normal((C, 1)) * 0.02).astype(np.float32),
        b_height=np.zeros(1, np.float32),
        W_cls=(rng.standard_normal((C, NC_CLS)) * 0.02).astype(np.float32),
        b_cls=np.zeros(NC_CLS, np.float32),
    )
    out = kernel(**ins)
    print("out", out.shape, out.dtype, float(np.abs(out).max()))


# revision 13
# speedup vs baseline: 1.1860x; 1.1860x over previous
"""Trainium2 Bass kernel for nn_CMTLiDAR (topk_masking).

Strategy (hardcoded for L=6, B=2, N=4096, C=512, NC=3, TOPK=120, 8 cores):

Only decoder layer -1 contributes to the output, so the 6-layer einsum
collapses to one layer.  Further, the center/height heads are only needed
for the 120 top-k rows per batch, so only the cls head runs on all rows.

Per-core roles: core c handles batch b = c//4, row chunk j = c%4
(rows [j*1024,(j+1)*1024) of that batch).

Pipeline per core:
  1. hidden2^T = relu(Wh2^T @ x^T + bh2)  (fp32 GEMM, rows sharded 4-way)
  2. cls logits -> softmax -> class-0 score for its 1024 rows
  3. AllGather scores within the 4-core batch group -> full 4096 scores
  4. kth_largest (gpsimd) -> exact 120th-largest threshold T
  5. mask + sparse_gather -> compacted candidate (idx, score) pairs
  6. pairwise rank (PE broadcast + DVE compares) with jax tie semantics
  7. indirect-DMA scatter by rank -> sorted top-120 (score, idx) in DRAM
  8. each core gathers its 30 output rows: x rows, ref points; computes
     center/height heads for those rows only; writes 30-row output slices
Host: shards/transposes inputs, concatenates the 8 core outputs.
"""

import numpy as np

import concourse.bass as bass
import concourse.bacc as bacc
import concourse.tile as tile
import concourse.mybir as mybir
from concourse.bass import ts, ds
from concourse.bass_utils import run_bass_kernel_spmd

F32 = mybir.dt.float32
I32 = mybir.dt.int32
U32 = mybir.dt.uint32
AF = mybir.ActivationFunctionType
OP = mybir.AluOpType

P = 128
C = 512
KT = C // P          # 4 k-tiles
B = 2
N = 4096
NC_CLS = 3
TOPK = 120
NCORES = 8
GROUP = 4            # cores per batch
RC = N // GROUP      # rows per core = 1024
OUTR = TOPK // GROUP  # output rows per core = 30

EPS = 1e-5
ONE_M_EPS = float(np.float32(1.0) - np.float32(1e-5))
NEG_BIG = -1.0e30


def emit(nc):
    """Emit the SPMD kernel body. Declares all DRAM I/O on nc."""
    # ---------------- inputs ----------------
    xT = nc.dram_tensor("xT", [C, RC], F32, kind="ExternalInput")
    xfull = nc.dram_tensor("xfull", [N, C], F32, kind="ExternalInput")
    refb = nc.dram_tensor("refb", [N, 3], F32, kind="ExternalInput")
    wh0 = nc.dram_tensor("wh0", [C, C], F32, kind="ExternalInput")
    wh1 = nc.dram_tensor("wh1", [C, C], F32, kind="ExternalInput")
    wh2 = nc.dram_tensor("wh2", [C, C], F32, kind="ExternalInput")
    bh0 = nc.dram_tensor("bh0", [C], F32, kind="ExternalInput")
    bh1 = nc.dram_tensor("bh1", [C], F32, kind="ExternalInput")
    bh2 = nc.dram_tensor("bh2", [C], F32, kind="ExternalInput")
    wcen = nc.dram_tensor("wcen", [C, 2], F32, kind="ExternalInput")
    bcen = nc.dram_tensor("bcen", [2], F32, kind="ExternalInput")
    whei = nc.dram_tensor("whei", [C, 1], F32, kind="ExternalInput")
    bhei = nc.dram_tensor("bhei", [1], F32, kind="ExternalInput")
    wcls = nc.dram_tensor("wcls", [C, NC_CLS], F32, kind="ExternalInput")
    bcls = nc.dram_tensor("bcls", [NC_CLS], F32, kind="ExternalInput")
    rowsel = nc.dram_tensor("rowsel", [OUTR, 1], I32, kind="ExternalInput")
    ident = nc.dram_tensor("ident", [P, P], F32, kind="ExternalInput")
    iota16 = nc.dram_tensor("iota16", [16, 256], F32, kind="ExternalInput")
    iotac0 = nc.dram_tensor("iotac0", [P, 1], F32, kind="ExternalInput")
    iotac1 = nc.dram_tensor("iotac1", [P, 1], F32, kind="ExternalInput")
    iotam = nc.dram_tensor("iotam", [P, 2 * P], F32, kind="ExternalInput")
    p16sel = nc.dram_tensor("p16sel", [16, P], F32, kind="ExternalInput")
    colsel = nc.dram_tensor("colsel", [P, 32], F32, kind="ExternalInput")
    iotar2 = nc.dram_tensor("iotar2", [1, 2 * P], F32, kind="ExternalInput")
    bselc = nc.dram_tensor("bselc", [P, 1], F32, kind="ExternalInput")
    bsel16 = nc.dram_tensor("bsel16", [16, 1], F32, kind="ExternalInput")
    # ---------------- outputs ----------------
    qry = nc.dram_tensor("qry", [OUTR, C], F32, kind="ExternalOutput")
    refq = nc.dram_tensor("refq", [OUTR, 3], F32, kind="ExternalOutput")
    cen = nc.dram_tensor("cen", [OUTR, 2], F32, kind="ExternalOutput")
    hei = nc.dram_tensor("hei", [OUTR, 1], F32, kind="ExternalOutput")
    sco = nc.dram_tensor("sco", [OUTR, 1], F32, kind="ExternalOutput")

    with tile.TileContext(nc) as tc:
        with tc.tile_pool(name="const", bufs=1) as cst, \
             tc.tile_pool(name="work", bufs=1) as wk, \
             tc.tile_pool(name="psum_mm", bufs=4, space="PSUM") as pmm, \
             tc.tile_pool(name="psum_sm", bufs=2, space="PSUM") as psm, \
             tc.tile_pool(name="dram", bufs=1, space="DRAM") as dr:

            # ======== load constants / weights ========
            ones1 = cst.tile([1, P], F32)
            nc.vector.memset(ones1[:], 1.0)
            ones_c = cst.tile([P, 1], F32)
            nc.vector.memset(ones_c[:], 1.0)

            # PE warm-up: small matmuls with no data deps to ramp the HAM
            # clock while the input DMAs stream in.
            warm_ps = psm.tile([P, 256], F32, tag="sm")
            for _ in range(8):
                nc.tensor.matmul(warm_ps[:, :P], lhsT=ones1[:], rhs=ones1[:],
                                 start=True, stop=True)
            warm_sb = wk.tile([1, 1], F32)
            nc.vector.tensor_copy(warm_sb[:], warm_ps[0:1, 0:1])
            warmd = dr.tile([1, 1], F32)
            nc.sync.dma_start(warmd[:], warm_sb[:])

            # warm-up collective: absorb the ncfw wakeup latency while the
            # GEMM phase runs, so the real AllGather starts hot
            wcin = dr.tile([8], F32)
            nc.sync.dma_start(wcin[:].rearrange("(o f) -> o f", o=1),
                              ones1[0:1, 0:8])
            wcout_t = nc.dram_tensor("wc_shared", [64], F32, kind="Internal",
                                     addr_space="Shared")
            nc.gpsimd.collective_compute(
                "AllGather", OP.bypass,
                replica_groups=[list(range(NCORES))],
                ins=[wcin.opt()], outs=[wcout_t.ap()])

            wh2_sb = cst.tile([P, KT, C], F32)
            xT_sb = wk.tile([P, KT, RC], F32)
            for ko in range(KT):
                for mo in range(KT):
                    nc.sync.dma_start(wh2_sb[:, ko, ts(mo, P)],
                                      wh2[ts(ko, P), ts(mo, P)])
                for no in range(RC // 512):
                    nc.scalar.dma_start(xT_sb[:, ko, ts(no, 512)],
                                        xT[ts(ko, P), ts(no, 512)])
            bh2_sb = cst.tile([P, KT], F32)
            nc.sync.dma_start(bh2_sb[:], bh2[:].rearrange("(mo p) -> p mo", p=P))
            wcls_sb = cst.tile([P, KT, NC_CLS], F32)
            nc.sync.dma_start(wcls_sb[:], wcls[:].rearrange("(ko p) n -> p ko n", p=P))
            bcls_sb = cst.tile([1, NC_CLS], F32)
            nc.sync.dma_start(bcls_sb[:], bcls[:].rearrange("(o n) -> o n", o=1))
            ident_sb = cst.tile([P, P], F32)
            nc.sync.dma_start(ident_sb[:], ident[:])

            # ======== stage 1: hidden2T = relu(Wh2^T x^T + bh2) ========
            h2T = wk.tile([P, KT, RC], F32)   # [d-part, d-tile, row]
            for mo in range(KT):
                pss = [pmm.tile([P, 512], F32, tag="mm", name=f"ps{mo}_{i}") for i in range(2)]
                for ko in range(KT):
                    for no in range(RC // 512):
                        nc.tensor.matmul(
                            pss[no][:],
                            lhsT=wh2_sb[:, ko, ts(mo, P)],
                            rhs=xT_sb[:, ko, ts(no, 512)],
                            start=(ko == 0), stop=(ko == KT - 1),
                        )
                for no in range(RC // 512):
                    dst = h2T[:, mo, ts(no, 512)]
                    if no % 2 == 0:
                        nc.scalar.activation(dst, pss[no][:], AF.Relu,
                                             bias=bh2_sb[:, mo:mo + 1])
                    else:
                        nc.vector.tensor_scalar(
                            out=dst, in0=pss[no][:],
                            scalar1=bh2_sb[:, mo:mo + 1], scalar2=0.0,
                            op0=OP.add, op1=OP.max)

            # ======== stage 1b: cls logits + softmax -> scores ========
            l_sb = wk.tile([P, 8, NC_CLS], F32)
            for ro in range(8):
                pl = psm.tile([P, 256], F32, tag="sm")
                for ko2 in range(KT):
                    nc.tensor.matmul(
                        pl[:, :NC_CLS],
                        lhsT=h2T[:, ko2, ts(ro, P)],
                        rhs=wcls_sb[:, ko2],
                        start=(ko2 == 0), stop=(ko2 == KT - 1))
                nc.vector.tensor_copy(l_sb[:, ro], pl[:, :NC_CLS])

            mx = wk.tile([P, 8], F32)
            nc.vector.tensor_reduce(mx[:], l_sb[:], axis=mybir.AxisListType.X,
                                    op=OP.max)
            lc = wk.tile([P, 8, NC_CLS], F32)
            nc.vector.tensor_tensor(
                out=lc[:], in0=l_sb[:],
                in1=mx[:, :, None].to_broadcast([P, 8, NC_CLS]),
                op=OP.subtract)
            bcb_ps = psm.tile([P, 256], F32, tag="sm")
            nc.tensor.matmul(bcb_ps[:, :NC_CLS], lhsT=ones1[:], rhs=bcls_sb[:],
                             start=True, stop=True)
            bcls_bc = wk.tile([P, NC_CLS], F32)
            nc.vector.tensor_copy(bcls_bc[:], bcb_ps[:, :NC_CLS])
            nc.vector.tensor_tensor(
                out=lc[:], in0=lc[:],
                in1=bcls_bc[:, None, :].to_broadcast([P, 8, NC_CLS]),
                op=OP.add)
            e_sb = wk.tile([P, 8, NC_CLS], F32)
            nc.scalar.activation(e_sb[:], lc[:], AF.Exp)
            ssum = wk.tile([P, 8], F32)
            nc.vector.tensor_reduce(ssum[:], e_sb[:], axis=mybir.AxisListType.X,
                                    op=OP.add)
            rs = wk.tile([P, 8], F32)
            nc.vector.reciprocal(rs[:], ssum[:])
            score_sb = wk.tile([P, 8], F32)
            nc.vector.tensor_tensor(out=score_sb[:], in0=e_sb[:, :, 0],
                                    in1=rs[:], op=OP.mult)

            # ======== stage 1c: allgather scores over batch group ========
            scl = dr.tile([RC], F32)
            nc.sync.dma_start(scl[:].rearrange("(ro p) -> p ro", p=P), score_sb[:])
            scg_t = nc.dram_tensor("scg_shared", [B * N], F32, kind="Internal",
                                   addr_space="Shared")
            scg = scg_t.ap()
            nc.gpsimd.collective_compute(
                "AllGather", OP.bypass,
                replica_groups=[list(range(NCORES))],
                ins=[scl.opt()], outs=[scg])

            # ======== stage 2: threshold = 120th largest of the union of
            # per-partition top-2 (a lower bound on the exact 120th-largest
            # score; the final ranker below restores exactness) ========
            s128b = wk.tile([P, B, N // P], F32)
            nc.sync.dma_start(s128b[:],
                              scg[:].rearrange("(b p f) -> p b f", b=B, p=P))
            s16b = wk.tile([16, B, N // 16], F32)
            nc.scalar.dma_start(s16b[:],
                                scg[:].rearrange("(b f p) -> p b f", b=B, p=16))
            bselc_sb = cst.tile([P, 1], F32)
            nc.sync.dma_start(bselc_sb[:], bselc[:])
            bsel16_sb = cst.tile([16, 1], F32)
            nc.sync.dma_start(bsel16_sb[:], bsel16[:])
            # blend the two batches: s = s0*(1-b) + s1*b = s0 + (s1-s0)*b
            s128 = wk.tile([P, N // P], F32)
            nc.vector.tensor_tensor(out=s128[:], in0=s128b[:, 1],
                                    in1=s128b[:, 0], op=OP.subtract)
            nc.vector.tensor_scalar(out=s128[:], in0=s128[:],
                                    scalar1=bselc_sb[:, 0:1], scalar2=None,
                                    op0=OP.mult)
            nc.vector.tensor_tensor(out=s128[:], in0=s128[:],
                                    in1=s128b[:, 0], op=OP.add)
            s16 = wk.tile([16, N // 16], F32)
            nc.vector.tensor_tensor(out=s16[:], in0=s16b[:, 1],
                                    in1=s16b[:, 0], op=OP.subtract)
            nc.vector.tensor_scalar(out=s16[:], in0=s16[:],
                                    scalar1=bsel16_sb[:, 0:1], scalar2=None,
                                    op0=OP.mult)
            nc.vector.tensor_tensor(out=s16[:], in0=s16[:],
                                    in1=s16b[:, 0], op=OP.add)
            iotam_sb = cst.tile([P, 2 * P], F32)
            nc.sync.dma_start(iotam_sb[:], iotam[:])
            iotac_sb = [cst.tile([P, 1], F32, tag=f"iotac{c}", name=f"iotac{c}")
                        for c in range(2)]
            nc.sync.dma_start(iotac_sb[0][:], iotac0[:])
            nc.sync.dma_start(iotac_sb[1][:], iotac1[:])
            # [m < j] tie-break masks, shared by both rankers
            mlt = []
            for c in range(2):
                ml = wk.tile([P, 2 * P], F32, tag=f"mlt{c}", name=f"mlt{c}")
                nc.vector.tensor_scalar(out=ml[:], in0=iotam_sb[:],
                                        scalar1=iotac_sb[c][:, 0:1],
                                        scalar2=None, op0=OP.is_lt)
                mlt.append(ml)

            m8 = wk.tile([P, 8], F32)
            nc.vector.max(m8[:], s128[:])
            # union candidate m = 128*f + p holds m8[p, f] (f in {0,1})
            u_row = wk.tile([1, 2 * P], F32)
            nc.sync.dma_start(u_row[:, 0:P], m8[:, 0:1])
            nc.scalar.dma_start(u_row[:, P:2 * P], m8[:, 1:2])
            bc2 = psm.tile([P, 512], F32, tag="sm")
            nc.tensor.matmul(bc2[:, :2 * P], lhsT=ones1[:], rhs=u_row[:],
                             start=True, stop=True)
            tv = []
            for c in range(2):
                eqc = wk.tile([P, 2 * P], F32, tag=f"eqc{c}", name=f"eqc{c}")
                nc.vector.scalar_tensor_tensor(
                    out=eqc[:], in0=bc2[:, :2 * P], scalar=m8[:, c:c + 1],
                    in1=mlt[c][:], op0=OP.is_equal, op1=OP.mult)
                gtc = wk.tile([P, 2 * P], F32, tag=f"gtc{c}", name=f"gtc{c}")
                rkc = wk.tile([P, 1], F32, tag=f"rkc{c}", name=f"rkc{c}")
                nc.vector.scalar_tensor_tensor(
                    out=gtc[:], in0=bc2[:, :2 * P], scalar=m8[:, c:c + 1],
                    in1=eqc[:], op0=OP.is_gt, op1=OP.add, accum_out=rkc[:])
                selc = wk.tile([P, 1], F32, tag=f"selc{c}", name=f"selc{c}")
                nc.vector.scalar_tensor_tensor(
                    out=selc[:], in0=rkc[:], scalar=float(TOPK - 1),
                    in1=m8[:, c:c + 1], op0=OP.is_equal, op1=OP.mult)
                tv.append(selc)
            nc.vector.tensor_tensor(out=tv[0][:], in0=tv[0][:], in1=tv[1][:],
                                    op=OP.add)
            t_ps = psm.tile([P, 256], F32, tag="sm")
            nc.tensor.matmul(t_ps[:1, :1], lhsT=tv[0][:], rhs=ones_c[:],
                             start=True, stop=True)
            t11 = wk.tile([1, 1], F32)
            nc.vector.tensor_copy(t11[:], t_ps[0:1, 0:1])
            t16p = psm.tile([P, 256], F32, tag="sm")
            nc.tensor.matmul(t16p[:16, :1], lhsT=ones1[0:1, 0:16], rhs=t11[:],
                             start=True, stop=True)
            t16 = wk.tile([16, 1], F32)
            nc.vector.tensor_copy(t16[:], t16p[:16, :1])

            # ======== stage 2b: mask + compact candidates (>= T) ========
            iota16_sb = cst.tile([16, 256], F32)
            nc.sync.dma_start(iota16_sb[:], iota16[:])
            neg16 = cst.tile([16, 256], F32)
            nc.vector.memset(neg16[:], -1.0)
            mask16 = wk.tile([16, 256], U32)
            nc.vector.tensor_scalar(out=mask16[:], in0=s16[:],
                                    scalar1=t16[:, 0:1], scalar2=None,
                                    op0=OP.is_ge)
            ms = wk.tile([16, 256], F32)
            nc.vector.select(ms[:], mask16[:], s16[:], neg16[:])
            mi = wk.tile([16, 256], F32)
            nc.vector.select(mi[:], mask16[:], iota16_sb[:], neg16[:])

            NCAND = 2 * P
            csco16 = wk.tile([16, 16], F32)
            nf1 = wk.tile([1, 1], U32)
            nc.gpsimd.sparse_gather(csco16[:], ms[:], num_found=nf1[:])
            cidx16 = wk.tile([16, 16], F32)
            nf2 = wk.tile([1, 1], U32)
            nc.gpsimd.sparse_gather(cidx16[:], mi[:], num_found=nf2[:])

            # score row [1, 256] (candidate j = p + 16f) via PE transpose +
            # one sbuf-to-sbuf reshape DMA
            tpc = psm.tile([P, 256], F32, tag="sm")
            nc.tensor.transpose(tpc[:16, :16], csco16[:], ident_sb[:16, :16])
            cts = wk.tile([16, 16], F32)
            nc.vector.tensor_copy(cts[:], tpc[:16, :16])
            cs_row = wk.tile([1, NCAND], F32)
            nc.sync.dma_start(cs_row[:], cts[:])

            # per-partition columns via PE one-hot select + masked reduce
            p16sel_sb = cst.tile([16, P], F32)
            nc.sync.dma_start(p16sel_sb[:], p16sel[:])
            colsel_sb = cst.tile([P, 32], F32)
            nc.sync.dma_start(colsel_sb[:], colsel[:])
            cs_col, ci_col = [], []
            for arr, src16 in ((0, csco16), (1, cidx16)):
                xp = psm.tile([P, 256], F32, tag="sm", name=f"xp{arr}")
                nc.tensor.matmul(xp[:, :16], lhsT=p16sel_sb[:], rhs=src16[:],
                                 start=True, stop=True)
                for c in range(2):
                    tm = wk.tile([P, 16], F32, tag=f"tm{arr}{c}",
                                 name=f"tm{arr}{c}")
                    nc.vector.tensor_tensor(
                        out=tm[:], in0=xp[:, :16],
                        in1=colsel_sb[:, ts(c, 16)], op=OP.mult)
                    col = wk.tile([P, 1], F32, tag=f"col{arr}{c}",
                                  name=f"col{arr}{c}")
                    nc.vector.tensor_reduce(col[:], tm[:],
                                            axis=mybir.AxisListType.X,
                                            op=OP.add)
                    (cs_col if arr == 0 else ci_col).append(col)

            # validity masks (slots beyond num_found hold garbage)
            iotar_sb = cst.tile([1, NCAND], F32)
            nc.sync.dma_start(iotar_sb[:], iotar2[:])
            nf11f = wk.tile([1, 1], F32)
            nc.vector.tensor_copy(nf11f[:], nf1[:])
            nfp = psm.tile([P, 256], F32, tag="sm")
            nc.tensor.matmul(nfp[:, :1], lhsT=ones1[:], rhs=nf11f[:],
                             start=True, stop=True)
            nf128f = wk.tile([P, 1], F32)
            nc.vector.tensor_copy(nf128f[:], nfp[:, :1])
            validr = wk.tile([1, NCAND], U32)
            nc.vector.tensor_scalar(out=validr[:], in0=iotar_sb[:],
                                    scalar1=nf11f[:, 0:1], scalar2=None,
                                    op0=OP.is_lt)
            negr = cst.tile([1, NCAND], F32)
            nc.vector.memset(negr[:], NEG_BIG)
            csm_row = wk.tile([1, NCAND], F32)
            nc.vector.select(csm_row[:], validr[:], cs_row[:], negr[:])
            negc = cst.tile([P, 1], F32)
            nc.vector.memset(negc[:], NEG_BIG)
            csm_col = []
            for c in range(2):
                validc = wk.tile([P, 1], U32, tag=f"validc{c}",
                                 name=f"validc{c}")
                nc.vector.tensor_tensor(out=validc[:], in0=iotac_sb[c][:],
                                        in1=nf128f[:], op=OP.is_lt)
                cm = wk.tile([P, 1], F32, tag=f"csmcol{c}", name=f"csmcol{c}")
                nc.vector.select(cm[:], validc[:], cs_col[c][:], negc[:])
                csm_col.append(cm)

            # broadcast scores across partitions; rank with jax tie
            # semantics ([m < j] on equal scores); scatter by rank
            bc = psm.tile([P, 512], F32, tag="sm")
            nc.tensor.matmul(bc[:, :NCAND], lhsT=ones1[:], rhs=csm_row[:],
                             start=True, stop=True)
            sortd = dr.tile([TOPK, 2], F32)
            for c in range(2):
                eq = wk.tile([P, NCAND], F32, tag=f"eq{c}", name=f"eq{c}")
                nc.vector.scalar_tensor_tensor(
                    out=eq[:], in0=bc[:, :NCAND], scalar=csm_col[c][:, 0:1],
                    in1=mlt[c][:], op0=OP.is_equal, op1=OP.mult)
                gt = wk.tile([P, NCAND], F32, tag=f"gt{c}", name=f"gt{c}")
                rankf = wk.tile([P, 1], F32, tag=f"rankf{c}", name=f"rankf{c}")
                nc.vector.scalar_tensor_tensor(
                    out=gt[:], in0=bc[:, :NCAND], scalar=csm_col[c][:, 0:1],
                    in1=eq[:], op0=OP.is_gt, op1=OP.add, accum_out=rankf[:])
                ranki = wk.tile([P, 1], I32, tag=f"ranki{c}", name=f"ranki{c}")
                nc.vector.tensor_copy(ranki[:], rankf[:])
                pairs = wk.tile([P, 2], F32, tag=f"pairs{c}", name=f"pairs{c}")
                nc.vector.tensor_copy(pairs[:, 0:1], cs_col[c][:])
                nc.vector.tensor_copy(pairs[:, 1:2], ci_col[c][:])
                nc.gpsimd.indirect_dma_start(
                    out=sortd[:],
                    out_offset=bass.IndirectOffsetOnAxis(ap=ranki[:, 0:1],
                                                         axis=0),
                    in_=pairs[:], in_offset=None,
                    bounds_check=TOPK - 1, oob_is_err=False)

            # ======== stage 3: gather this core's 30 rows ========
            rsel = wk.tile([OUTR, 1], I32)
            nc.sync.dma_start(rsel[:], rowsel[:])
            pairs30 = wk.tile([OUTR, 2], F32)
            nc.gpsimd.indirect_dma_start(
                out=pairs30[:], out_offset=None,
                in_=sortd[:],
                in_offset=bass.IndirectOffsetOnAxis(ap=rsel[:, 0:1], axis=0))
            idx30 = wk.tile([OUTR, 1], I32)
            nc.vector.tensor_copy(idx30[:], pairs30[:, 1:2])
            xg = wk.tile([OUTR, C], F32)
            nc.gpsimd.indirect_dma_start(
                out=xg[:], out_offset=None,
                in_=xfull[:],
                in_offset=bass.IndirectOffsetOnAxis(ap=idx30[:, 0:1], axis=0))
            refg = wk.tile([OUTR, 3], F32)
            nc.gpsimd.indirect_dma_start(
                out=refg[:], out_offset=None,
                in_=refb[:],
                in_offset=bass.IndirectOffsetOnAxis(ap=idx30[:, 0:1], axis=0))

            # ======== stage 4: center/height heads on 30 rows ========
            xgT = wk.tile([P, KT, OUTR], F32)
            for kt in range(KT):
                tp = psm.tile([P, 256], F32, tag="sm")
                nc.tensor.transpose(tp[:, :OUTR], xg[:, ts(kt, P)],
                                    ident_sb[:OUTR, :OUTR])
                nc.vector.tensor_copy(xgT[:, kt], tp[:, :OUTR])

            hT = []
            for h, (whd, bhd) in enumerate([(wh0, bh0), (wh1, bh1)]):
                wh_sb = cst.tile([P, KT, C], F32, tag=f"wh{h}sb")
                nc.sync.dma_start(wh_sb[:],
                                  whd[:].rearrange("(ko p) d -> p ko d", p=P))
                bh_sb = cst.tile([P, KT], F32, tag=f"bh{h}sb")
                nc.sync.dma_start(bh_sb[:],
                                  bhd[:].rearrange("(mo p) -> p mo", p=P))
                hT_h = wk.tile([P, KT, OUTR], F32, tag=f"hT{h}")
                for mo in range(KT):
                    hp = psm.tile([P, 256], F32, tag="sm")
                    for ko in range(KT):
                        nc.tensor.matmul(
                            hp[:, :OUTR],
                            lhsT=wh_sb[:, ko, ts(mo, P)],
                            rhs=xgT[:, ko],
                            start=(ko == 0), stop=(ko == KT - 1))
                    nc.scalar.activation(hT_h[:, mo], hp[:, :OUTR], AF.Relu,
                                         bias=bh_sb[:, mo:mo + 1])
                hT.append(hT_h)

            wcen_sb = cst.tile([P, KT, 2], F32)
            nc.sync.dma_start(wcen_sb[:], wcen[:].rearrange("(ko p) n -> p ko n", p=P))
            bcen_sb = cst.tile([1, 2], F32)
            nc.sync.dma_start(bcen_sb[:], bcen[:].rearrange("(o n) -> o n", o=1))
            whei_sb = cst.tile([P, KT, 1], F32)
            nc.sync.dma_start(whei_sb[:], whei[:].rearrange("(ko p) n -> p ko n", p=P))
            bhei_sb = cst.tile([1, 1], F32)
            nc.sync.dma_start(bhei_sb[:], bhei[:].rearrange("(o n) -> o n", o=1))
            ones30 = cst.tile([1, OUTR], F32)
            nc.vector.memset(ones30[:], 1.0)

            cen_ps = psm.tile([P, 256], F32, tag="sm")
            for ko2 in range(KT):
                nc.tensor.matmul(cen_ps[:OUTR, :2], lhsT=hT[0][:, ko2],
                                 rhs=wcen_sb[:, ko2],
                                 start=(ko2 == 0), stop=False)
            nc.tensor.matmul(cen_ps[:OUTR, :2], lhsT=ones30[:], rhs=bcen_sb[:],
                             start=False, stop=True)
            hei_ps = psm.tile([P, 256], F32, tag="sm")
            for ko2 in range(KT):
                nc.tensor.matmul(hei_ps[:OUTR, :1], lhsT=hT[1][:, ko2],
                                 rhs=whei_sb[:, ko2],
                                 start=(ko2 == 0), stop=False)
            nc.tensor.matmul(hei_ps[:OUTR, :1], lhsT=ones30[:], rhs=bhei_sb[:],
                             start=False, stop=True)

            # ======== stage 5: epilogue ========
            rcl = wk.tile([OUTR, 3], F32)
            nc.vector.tensor_scalar(out=rcl[:], in0=refg[:],
                                    scalar1=EPS, scalar2=ONE_M_EPS,
                                    op0=OP.max, op1=OP.min)
            ln_r = wk.tile([OUTR, 3], F32)
            nc.scalar.activation(ln_r[:], rcl[:], AF.Ln)
            om = wk.tile([OUTR, 3], F32)
            nc.vector.tensor_scalar(out=om[:], in0=rcl[:],
                                    scalar1=-1.0, scalar2=1.0,
                                    op0=OP.mult, op1=OP.add)
            ln_om = wk.tile([OUTR, 3], F32)
            nc.scalar.activation(ln_om[:], om[:], AF.Ln)
            iv = wk.tile([OUTR, 3], F32)
            nc.vector.tensor_sub(iv[:], ln_r[:], ln_om[:])

            cr = wk.tile([OUTR, 2], F32)
            nc.vector.tensor_add(cr[:], cen_ps[:OUTR, :2], iv[:, 0:2])
            csg = wk.tile([OUTR, 2], F32)
            nc.scalar.activation(csg[:], cr[:], AF.Sigmoid)
            cxy = wk.tile([OUTR, 2], F32)
            nc.vector.tensor_scalar(out=cxy[:], in0=csg[:],
                                    scalar1=108.0, scalar2=-54.0,
                                    op0=OP.mult, op1=OP.add)
            hr = wk.tile([OUTR, 1], F32)
            nc.vector.tensor_add(hr[:], hei_ps[:OUTR, :1], iv[:, 2:3])
            hsg = wk.tile([OUTR, 1], F32)
            nc.scalar.activation(hsg[:], hr[:], AF.Sigmoid)
            hz = wk.tile([OUTR, 1], F32)
            nc.vector.tensor_scalar(out=hz[:], in0=hsg[:],
                                    scalar1=8.0, scalar2=-5.0,
                                    op0=OP.mult, op1=OP.add)

            # ======== outputs ========
            nc.sync.dma_start(qry[:], xg[:])
            nc.sync.dma_start(refq[:], refg[:])
            nc.sync.dma_start(cen[:], cxy[:])
            nc.sync.dma_start(hei[:], hz[:])
            nc.sync.dma_start(sco[:], pairs30[:, 0:1])


_CACHE = {}


def build():
    if "nc" not in _CACHE:
        nc = bacc.Bacc("TRN2", target_bir_lowering=False, debug=False,
                       num_devices=NCORES)
        emit(nc)
        nc.compile()
        _CACHE["nc"] = nc
    return _CACHE["nc"]


def make_in_maps(outs_dec, reference_points, Wh, bh, W_center, b_center,
                 W_height, b_height, W_cls, b_cls):
    x = np.ascontiguousarray(outs_dec[-1])          # [B, N, C]
    iota16 = (np.arange(256, dtype=np.float32)[None, :] * 16
              + np.arange(16, dtype=np.float32)[:, None])
    common = dict(
        wh0=np.ascontiguousarray(Wh[0]), wh1=np.ascontiguousarray(Wh[1]),
        wh2=np.ascontiguousarray(Wh[2]),
        bh0=np.ascontiguousarray(bh[0]), bh1=np.ascontiguousarray(bh[1]),
        bh2=np.ascontiguousarray(bh[2]),
        wcen=np.ascontiguousarray(W_center), bcen=np.ascontiguousarray(b_center),
        whei=np.ascontiguousarray(W_height), bhei=np.ascontiguousarray(b_height),
        wcls=np.ascontiguousarray(W_cls), bcls=np.ascontiguousarray(b_cls),
        ident=np.eye(P, dtype=np.float32),
        iota16=np.ascontiguousarray(iota16),
        iotac0=np.arange(P, dtype=np.float32)[:, None].copy(),
        iotac1=(P + np.arange(P, dtype=np.float32))[:, None].copy(),
        iotam=np.tile(np.arange(2 * P, dtype=np.float32), (P, 1)),
        p16sel=(np.arange(16)[:, None] == (np.arange(P)[None, :] % 16)
                ).astype(np.float32),
        colsel=np.concatenate(
            [(np.arange(16)[None, :] == (8 * c + np.arange(P)[:, None] // 16)
              ).astype(np.float32) for c in range(2)], axis=1),
        iotar2=np.arange(2 * P, dtype=np.float32)[None, :].copy(),
    )
    in_maps = []
    for c in range(NCORES):
        b, j = divmod(c, GROUP)
        xb = x[b]
        in_maps.append(dict(
            common,
            xT=np.ascontiguousarray(xb[j * RC:(j + 1) * RC].T),
            xfull=xb,
            refb=np.ascontiguousarray(reference_points[b]),
            rowsel=(j * OUTR + np.arange(OUTR, dtype=np.int32))[:, None].copy(),
            bselc=np.full((P, 1), float(b), np.float32),
            bsel16=np.full((16, 1), float(b), np.float32),
        ))
    return in_maps


def assemble(results):
    out = np.zeros((B, TOPK, C + 3 + 3 + 1), dtype=np.float32)
    for c in range(NCORES):
        b, j = divmod(c, GROUP)
        r = results[c]
        sl = slice(j * OUTR, (j + 1) * OUTR)
        out[b, sl, :C] = r["qry"]
        out[b, sl, C:C + 3] = r["refq"]
        out[b, sl, C + 3:C + 5] = r["cen"]
        out[b, sl, C + 5:C + 6] = r["hei"]
        out[b, sl, C + 6] = r["sco"][:, 0]
    return out


def kernel(**inputs):
    nc = build()
    in_maps = make_in_maps(**{k: np.asarray(v) for k, v in inputs.items()})
    res = run_bass_kernel_spmd(nc, in_maps, core_ids=list(range(NCORES)))
    return assemble(res.results)


if __name__ == "__main__":
    rng = np.random.default_rng(0)
    ins = dict(
        outs_dec=rng.standard_normal((6, B, N, C)).astype(np.float32),
        reference_points=rng.uniform(0.01, 0.99, (B, N, 3)).astype(np.float32),
        Wh=(rng.standard_normal((3, C, C)) * 0.02).astype(np.float32),
        bh=np.zeros((3, C), np.float32),
        W_center=(rng.standard_normal((C, 2)) * 0.02).astype(np.float32),
        b_center=np.zeros(2, np.float32),
        W_height=(rng.standard_normal((C, 1)) * 0.02).astype(np.float32),
        b_height=np.zeros(1, np.float32),
        W_cls=(rng.standard_normal((C, NC_CLS)) * 0.02).astype(np.float32),
        b_cls=np.zeros(NC_CLS, np.float32),
    )
    out = kernel(**ins)
    print("out", out.shape, out.dtype, float(np.abs(out).max()))
